# revision 11
# baseline (speedup 1.0000x reference)
"""Trainium2 Bass kernel for shifted-query cross-attention.

Problem: x [B=4, N=2048, D=512], W_qkv [3*H*DH=1536, D]; H=8 heads, DH=64.
  qkv = x @ W.T; q,k,v per head; q_cross[t] = q[t-1] (q_cross[0]=q[0]);
  out = softmax(q_cross*scale @ k.T) @ v, heads re-concatenated.

Sharding: 8 cores = 4 batches x 2 head-groups (4 heads each). Communication
free: each core gets x[b] and the W rows of its 4 heads, produces
out[b][:, g*256:(g+1)*256].

Per-core plan (matmul inputs bf16 -- 1 cyc/row on the PE vs 2 for fp32r --
with fp32 accumulation in PSUM throughout):
  - transpose x -> xT [D, N] and W-shard -> wT [D, 768] via fp32 TensorE
    identity transposes; the PSUM->SBUF copy casts to bf16.
  - proj: qT/kT feature-major [256, N] = wT.T @ xT  (heads pair-packed:
    chunk hp holds head 2hp in partitions 0-63, head 2hp+1 in 64-127);
    v token-major [N, 256] = xT.T @ wT_v, stored with a 65th all-ones
    column per head ([v_h | 1]).
  - scores transposed: ST[j, i] = kT.T @ qcT per 128-token j-chunk, two
    heads row-packed on the PE array (K=64 each, array rows 0-63/64-127).
  - exp on ScalarE straight from PSUM with scale folded in (no max
    subtraction: |scores*scale| <= ~2, exp is safe in fp32).
  - outT[65, i] += [v_h | 1].T @ E accumulated over j-chunks in PSUM;
    row 64 = softmax denominators (free via the ones column).
  - PE-transpose outT back to token-major (fp32), multiply by reciprocal
    row sums on VectorE, DMA out.

The kernel is ScalarE-bound (134M softmax exps at 1 elem/lane/cycle), so
the schedule keeps ACT saturated: all PSUM pools coexist (no inter-phase
barriers), attention for head pair 0 starts as soon as its q/k and v
projections land, head pair 1's projection is interleaved into head pair
0's attention (PE slack), and scores run one j-chunk ahead of attn@v.
"""

import os
import sys

for _p in ("/opt/trn_rl_repo", "/root/.axon_site/_ro/trn_rl_repo"):
    if os.path.isdir(_p) and _p not in sys.path:
        sys.path.append(_p)

from contextlib import ExitStack

import numpy as np

import concourse.bass as bass
import concourse.tile as tile
from concourse import bacc, mybir
from concourse.masks import make_identity

B, N, D = 4, 2048, 512
H, DH = 8, 64
SCALE = DH**-0.5
NCORES = 8
HG = H // 2  # heads per core = 4
RV = HG * DH  # 256 v rows / output cols per core

F32 = mybir.dt.float32
BF16 = mybir.dt.bfloat16
EXP = mybir.ActivationFunctionType.Exp


def build_kernel(nc: bass.Bass, n_tok: int = N):
    """Emit the per-core program. n_tok parameterized for small sim tests."""
    NI = n_tok // 128  # 128-token chunks
    NI5 = n_tok // 512  # 512-token chunks

    x_d = nc.dram_tensor("x", [n_tok, D], F32, kind="ExternalInput").ap()
    w_d = nc.dram_tensor("w", [3 * RV, D], F32, kind="ExternalInput").ap()
    o_d = nc.dram_tensor("o", [n_tok, RV], F32, kind="ExternalOutput").ap()

    # pre-attention copies alternate between VectorE and the still-idle
    # ScalarE; once exp streaming starts everything goes to VectorE
    _flip = [0]

    def precopy(dst, src):
        _flip[0] ^= 1
        if _flip[0]:
            nc.vector.tensor_copy(dst, src)
        else:
            nc.scalar.copy(dst, src)

    with tile.TileContext(nc) as tc, ExitStack() as ctx:
        cpool = ctx.enter_context(tc.tile_pool(name="const", bufs=1))
        identf = cpool.tile([128, 128], F32)
        make_identity(nc, identf[:])

        sb = ctx.enter_context(tc.tile_pool(name="persist", bufs=1))
        w_sb = sb.tile([128, 6, D], F32)
        wT = sb.tile([128, 4, 3 * RV], BF16)  # wT[p, kc, r] = w[r, kc*128+p]
        x_sb = sb.tile([128, NI, D], F32)
        xT = sb.tile([128, 4, n_tok], BF16)  # xT[p, kc, i] = x[i, kc*128+p]
        # qcT[p, hp, 1+t] = q[t] for head pair hp; col 0 duplicates q[0]
        qcT = sb.tile([128, 2, n_tok + 8], BF16)
        kT = sb.tile([128, 2, n_tok], BF16)
        # v1[p, jc, ha*65 + dd] = v[jc*128+p, ha*64+dd] for dd<64; 1.0 at dd=64
        v1 = sb.tile([128, NI, HG * 65], BF16)
        out_sb = sb.tile([128, NI, RV], F32)

        ones_f = sb.tile([128, NI, HG, 1], F32)
        nc.vector.memset(ones_f[:], 1.0)
        nc.vector.tensor_copy(
            v1[:].rearrange("p n (h e) -> p n h e", e=65)[:, :, :, 64:65],
            ones_f[:],
        )

        # all PSUM pools coexist: st 4 banks + ot/tro 2 + pt/pp 2 = 8
        ps512 = ctx.enter_context(tc.tile_pool(name="ps512", bufs=2, space="PSUM"))
        ps_st = ctx.enter_context(tc.tile_pool(name="ps_st", bufs=2, space="PSUM"))
        ps_ot = ctx.enter_context(tc.tile_pool(name="ps_ot", bufs=2, space="PSUM"))
        e_pool = ctx.enter_context(tc.tile_pool(name="e_sb", bufs=3))
        ot_pool = ctx.enter_context(tc.tile_pool(name="ot_sb", bufs=2))
        rs_pool = ctx.enter_context(tc.tile_pool(name="rs", bufs=4))

        # ---- loads (two DMA queues) + fp32 transposes, cast on copy-out ----
        for rc in range(6):
            nc.sync.dma_start(w_sb[:, rc, :], w_d[rc * 128 : (rc + 1) * 128, :])
        for ic in range(NI):
            q = nc.sync if ic % 2 == 0 else nc.gpsimd
            q.dma_start(x_sb[:, ic, :], x_d[ic * 128 : (ic + 1) * 128, :])

        # batch 4 transposes into one PSUM bank + one wide copy so each
        # downstream matmul operand slice has a single producer (walrus
        # caps sync-waits per lowered matmul instruction)
        for kc in range(4):
            for rg in range(2):  # w row-chunk groups: 0-3 and 4-5
                rcs = range(4) if rg == 0 else range(4, 6)
                pt = ps512.tile([128, 512], F32, tag="ps512", name=f"ptw{kc}_{rg}")
                for t, rc in enumerate(rcs):
                    nc.tensor.transpose(
                        pt[:, t * 128 : (t + 1) * 128],
                        w_sb[:, rc, kc * 128 : (kc + 1) * 128],
                        identf[:],
                    )
                nw = len(rcs) * 128
                precopy(wT[:, kc, rg * 512 : rg * 512 + nw], pt[:, :nw])
        for kc in range(4):
            for i5 in range(NI5):
                pt = ps512.tile([128, 512], F32, tag="ps512", name=f"ptx{kc}_{i5}")
                for t in range(4):
                    nc.tensor.transpose(
                        pt[:, t * 128 : (t + 1) * 128],
                        x_sb[:, i5 * 4 + t, kc * 128 : (kc + 1) * 128],
                        identf[:],
                    )
                precopy(xT[:, kc, i5 * 512 : (i5 + 1) * 512], pt[:])

        # ---- projections ----
        def proj_qk(m, i5):
            """m = 0,1: q head pairs (0,1),(2,3); m = 2,3: k head pairs."""
            pp = ps512.tile([128, 512], F32, tag="ps512", name=f"pp{m}_{i5}")
            for kc in range(4):
                nc.tensor.matmul(
                    pp[:],
                    wT[:, kc, m * 128 : (m + 1) * 128],
                    xT[:, kc, i5 * 512 : (i5 + 1) * 512],
                    start=(kc == 0),
                    stop=(kc == 3),
                )
            hp = m % 2
            if m < 2:
                nc.vector.tensor_copy(
                    qcT[:, hp, 1 + i5 * 512 : 1 + (i5 + 1) * 512], pp[:]
                )
                if i5 == 0:
                    nc.vector.tensor_copy(qcT[:, hp, 0:1], pp[:, 0:1])
            else:
                nc.vector.tensor_copy(kT[:, hp, i5 * 512 : (i5 + 1) * 512], pp[:])

        def proj_v(jc):
            pv = ps512.tile([128, 512], F32, tag="ps512", name=f"pv{jc}")
            for kc in range(4):
                nc.tensor.matmul(
                    pv[:, :RV],
                    xT[:, kc, jc * 128 : (jc + 1) * 128],
                    wT[:, kc, 2 * RV : 3 * RV],
                    start=(kc == 0),
                    stop=(kc == 3),
                )
            precopy(
                v1[:, jc, :].rearrange("p (h e) -> p h e", e=65)[:, :, 0:64],
                pv[:, :RV].rearrange("p (h e) -> p h e", e=64),
            )

        # head pair 0's q/k, then v (needed by both pairs)
        for i5 in range(NI5):
            proj_qk(0, i5)  # wait: m=0 is q of hp 0
        for i5 in range(NI5):
            proj_qk(2, i5)  # k of hp 0
        for jc in range(NI):
            proj_v(jc)

        # ---- attention ----
        def attn_block(hp, i5, extra=None):
            """One (head pair, 512-token i-chunk): scores pipelined one
            j-chunk ahead of attn@v; `extra` emits interleaved PE work."""
            ot_ps = [
                ps_ot.tile([65, 512], F32, tag="ot", name=f"ot{hp}_{i5}_{h}")
                for h in range(2)
            ]
            ets = [None] * NI

            def scores(jc):
                st = ps_st.tile([128, 1024], F32, tag="st", name=f"st{hp}_{i5}_{jc}")
                nc.tensor.matmul(
                    st[:, 0:512],
                    kT[0:64, hp, jc * 128 : (jc + 1) * 128],
                    qcT[0:64, hp, i5 * 512 : (i5 + 1) * 512],
                    start=True,
                    stop=True,
                    tile_position=(0, 0),
                )
                nc.tensor.matmul(
                    st[:, 512:1024],
                    kT[64:128, hp, jc * 128 : (jc + 1) * 128],
                    qcT[64:128, hp, i5 * 512 : (i5 + 1) * 512],
                    start=True,
                    stop=True,
                    tile_position=(64, 0),
                )
                et = e_pool.tile([128, 1024], BF16, tag="et", name=f"et{hp}_{i5}_{jc}")
                nc.scalar.activation(et[:], st[:], EXP, scale=SCALE)
                ets[jc] = et

            def attnv(jc):
                for h in range(2):
                    ha = hp * 2 + h
                    nc.tensor.matmul(
                        ot_ps[h][:],
                        v1[:, jc, ha * 65 : (ha + 1) * 65],
                        ets[jc][:, h * 512 : (h + 1) * 512],
                        start=(jc == 0),
                        stop=(jc == NI - 1),
                    )

            scores(0)
            for jc in range(1, NI):
                scores(jc)
                attnv(jc - 1)
            attnv(NI - 1)
            if extra is not None:
                extra()

            for h in range(2):
                ha = hp * 2 + h
                ots = ot_pool.tile([65, 512], F32, tag="ots", name=f"ots{hp}_{i5}_{h}")
                nc.vector.tensor_copy(ots[:], ot_ps[h][:])
                for t in range(4):
                    ic = i5 * 4 + t
                    tr = ps_ot.tile([128, 65], F32, tag="ot", name=f"tr{hp}_{i5}_{h}_{t}")
                    nc.tensor.transpose(
                        tr[:], ots[:, t * 128 : (t + 1) * 128], identf[0:65, 0:65]
                    )
                    rs = rs_pool.tile([128, 1], F32, tag="rs", name=f"rs{hp}_{i5}_{h}_{t}")
                    nc.vector.reciprocal(rs[:], tr[:, 64:65])
                    nc.vector.tensor_scalar_mul(
                        out_sb[:, ic, ha * 64 : (ha + 1) * 64],
                        tr[:, 0:64],
                        rs[:],
                    )

        # head pair 0's attention, with head pair 1's projection interleaved
        # into the PE slack left by the ACT-bound exp stream
        for i5 in range(NI5):

            def extra(i5=i5):
                proj_qk(1, i5)
                proj_qk(3, i5)

            attn_block(0, i5, extra=extra)

        for i5 in range(NI5):
            attn_block(1, i5)
            for t in range(4):
                ic = i5 * 4 + t
                nc.sync.dma_start(o_d[ic * 128 : (ic + 1) * 128, :], out_sb[:, ic, :])

    return nc


def make_nc(n_tok: int = N) -> bass.Bass:
    nc = bacc.Bacc("TRN2", target_bir_lowering=False, debug=False)
    build_kernel(nc, n_tok=n_tok)
    nc.compile()
    return nc


def shard_inputs(x: np.ndarray, W_qkv: np.ndarray) -> list[dict]:
    """Core c = (b, g): b = c // 2, g = c % 2 (heads 4g..4g+3)."""
    in_maps = []
    for c in range(NCORES):
        b, g = divmod(c, 2)
        r0 = g * RV
        w_shard = np.concatenate(
            [
                W_qkv[r0 : r0 + RV],
                W_qkv[512 + r0 : 512 + r0 + RV],
                W_qkv[1024 + r0 : 1024 + r0 + RV],
            ],
            axis=0,
        )
        in_maps.append(
            {
                "x": np.ascontiguousarray(x[b], dtype=np.float32),
                "w": np.ascontiguousarray(w_shard, dtype=np.float32),
            }
        )
    return in_maps


def gather_outputs(results: list[dict]) -> np.ndarray:
    out = np.empty((B, N, H * DH), dtype=np.float32)
    for c in range(NCORES):
        b, g = divmod(c, 2)
        out[b, :, g * RV : (g + 1) * RV] = results[c]["o"]
    return out


_CACHED_NC = None


def kernel(x: np.ndarray, W_qkv: np.ndarray) -> np.ndarray:
    global _CACHED_NC
    from concourse.bass_utils import run_bass_kernel_spmd

    if _CACHED_NC is None:
        _CACHED_NC = make_nc()
    in_maps = shard_inputs(np.asarray(x), np.asarray(W_qkv))
    res = run_bass_kernel_spmd(_CACHED_NC, in_maps, core_ids=list(range(NCORES)))
    return gather_outputs(res.results)


if __name__ == "__main__":
    rng = np.random.default_rng(0)
    x = rng.standard_normal((B, N, D), dtype=np.float32)
    w = (rng.standard_normal((3 * H * DH, D), dtype=np.float32) * 0.02).astype(
        np.float32
    )
    out = kernel(x, w)
    print(out.shape, out.dtype)


# revision 14
# speedup vs baseline: 1.1606x; 1.1606x over previous
"""Trainium2 Bass kernel for shifted-query cross-attention.

Problem: x [B=4, N=2048, D=512], W_qkv [3*H*DH=1536, D]; H=8 heads, DH=64.
  qkv = x @ W.T; q,k,v per head; q_cross[t] = q[t-1] (q_cross[0]=q[0]);
  out = softmax(q_cross*scale @ k.T) @ v, heads re-concatenated.

Sharding: 8 cores = 4 batches x 2 head-groups (4 heads each). Communication
free: each core gets x[b] and the W rows of its 4 heads, produces
out[b][:, g*256:(g+1)*256].

Per-core plan (matmul inputs bf16 -- 1 cyc/row on the PE vs 2 for fp32r --
with fp32 accumulation in PSUM throughout):
  - transpose x -> xT [D, N] and W-shard -> wT [D, 768] via fp32 TensorE
    identity transposes; the PSUM->SBUF copy casts to bf16.
  - proj: qT/kT feature-major [256, N] = wT.T @ xT  (heads pair-packed:
    chunk hp holds head 2hp in partitions 0-63, head 2hp+1 in 64-127);
    v token-major [N, 256] = xT.T @ wT_v, stored with a 65th all-ones
    column per head ([v_h | 1]).
  - scores transposed: ST[j, i] = kT.T @ qcT per 128-token j-chunk, two
    heads row-packed on the PE array (K=64 each, array rows 0-63/64-127).
  - exp on ScalarE straight from PSUM with scale folded in (no max
    subtraction: |scores*scale| <= ~2, exp is safe in fp32).
  - outT[65, i] += [v_h | 1].T @ E accumulated over j-chunks in PSUM;
    row 64 = softmax denominators (free via the ones column).
  - PE-transpose outT back to token-major (fp32), multiply by reciprocal
    row sums on VectorE, DMA out.

The kernel is ScalarE-bound (134M softmax exps at 1 elem/lane/cycle), so
the schedule keeps ACT saturated: all PSUM pools coexist (no inter-phase
barriers), attention for head pair 0 starts as soon as its q/k and v
projections land, head pair 1's projection is interleaved into head pair
0's attention (PE slack), and scores run one j-chunk ahead of attn@v.
"""

import os
import sys

for _p in ("/opt/trn_rl_repo", "/root/.axon_site/_ro/trn_rl_repo"):
    if os.path.isdir(_p) and _p not in sys.path:
        sys.path.append(_p)

from contextlib import ExitStack

import numpy as np

import concourse.bass as bass
import concourse.tile as tile
from concourse import bacc, mybir
from concourse.masks import make_identity

B, N, D = 4, 2048, 512
H, DH = 8, 64
SCALE = DH**-0.5
NCORES = 8
HG = H // 2  # heads per core = 4
RV = HG * DH  # 256 v rows / output cols per core

F32 = mybir.dt.float32
BF16 = mybir.dt.bfloat16
EXP = mybir.ActivationFunctionType.Exp


def build_kernel(nc: bass.Bass, n_tok: int = N):
    """Emit the per-core program. n_tok parameterized for small sim tests."""
    NI = n_tok // 128  # 128-token chunks
    NI5 = n_tok // 512  # 512-token chunks

    x_d = nc.dram_tensor("x", [n_tok, D], F32, kind="ExternalInput").ap()
    w_d = nc.dram_tensor("w", [3 * RV, D], F32, kind="ExternalInput").ap()
    o_d = nc.dram_tensor("o", [n_tok, RV], F32, kind="ExternalOutput").ap()

    # pre-attention copies alternate between VectorE and the still-idle
    # ScalarE; once exp streaming starts everything goes to VectorE
    _flip = [0]

    def precopy(dst, src):
        _flip[0] ^= 1
        if _flip[0]:
            nc.vector.tensor_copy(dst, src)
        else:
            nc.scalar.copy(dst, src)

    with tile.TileContext(nc) as tc, ExitStack() as ctx:
        cpool = ctx.enter_context(tc.tile_pool(name="const", bufs=1))
        identf = cpool.tile([128, 128], F32)
        make_identity(nc, identf[:])

        sb = ctx.enter_context(tc.tile_pool(name="persist", bufs=1))
        w_sb = sb.tile([128, 6, D], F32)
        wT = sb.tile([128, 4, 3 * RV], BF16)  # wT[p, kc, r] = w[r, kc*128+p]
        x_sb = sb.tile([128, NI, D], F32)
        xT = sb.tile([128, 4, n_tok], BF16)  # xT[p, kc, i] = x[i, kc*128+p]
        # qcT[p, hp, 1+t] = q[t] for head pair hp; col 0 duplicates q[0]
        qcT = sb.tile([128, 2, n_tok + 8], BF16)
        kT = sb.tile([128, 2, n_tok], BF16)
        # v1[p, jc, ha*65 + dd] = v[jc*128+p, ha*64+dd] for dd<64; 1.0 at dd=64
        v1 = sb.tile([128, NI, HG * 65], BF16)
        out_sb = sb.tile([128, NI, RV], F32)

        ones_f = sb.tile([128, NI, HG, 1], F32)
        nc.vector.memset(ones_f[:], 1.0)
        nc.vector.tensor_copy(
            v1[:].rearrange("p n (h e) -> p n h e", e=65)[:, :, :, 64:65],
            ones_f[:],
        )

        # all PSUM pools coexist: st 4 banks + ot/tro 2 + pt/pp 2 = 8
        ps512 = ctx.enter_context(tc.tile_pool(name="ps512", bufs=2, space="PSUM"))
        ps_st = ctx.enter_context(tc.tile_pool(name="ps_st", bufs=2, space="PSUM"))
        ps_ot = ctx.enter_context(tc.tile_pool(name="ps_ot", bufs=2, space="PSUM"))
        e_pool = ctx.enter_context(tc.tile_pool(name="e_sb", bufs=4))
        ot_pool = ctx.enter_context(tc.tile_pool(name="ot_sb", bufs=2))
        rs_pool = ctx.enter_context(tc.tile_pool(name="rs", bufs=4))

        # ---- loads (two DMA queues) + fp32 transposes, cast on copy-out ----
        for rc in range(6):
            nc.sync.dma_start(w_sb[:, rc, :], w_d[rc * 128 : (rc + 1) * 128, :])
        for ic in range(NI):
            q = nc.sync if ic % 2 == 0 else nc.gpsimd
            q.dma_start(x_sb[:, ic, :], x_d[ic * 128 : (ic + 1) * 128, :])

        # batch 4 transposes into one PSUM bank + one wide copy so each
        # downstream matmul operand slice has a single producer (walrus
        # caps sync-waits per lowered matmul instruction)
        for kc in range(4):
            for rg in range(2):  # w row-chunk groups: 0-3 and 4-5
                rcs = range(4) if rg == 0 else range(4, 6)
                pt = ps512.tile([128, 512], F32, tag="ps512", name=f"ptw{kc}_{rg}")
                for t, rc in enumerate(rcs):
                    nc.tensor.transpose(
                        pt[:, t * 128 : (t + 1) * 128],
                        w_sb[:, rc, kc * 128 : (kc + 1) * 128],
                        identf[:],
                    )
                nw = len(rcs) * 128
                precopy(wT[:, kc, rg * 512 : rg * 512 + nw], pt[:, :nw])

        def transpose_x(i5):
            for kc in range(4):
                pt = ps512.tile([128, 512], F32, tag="ps512", name=f"ptx{kc}_{i5}")
                for t in range(4):
                    nc.tensor.transpose(
                        pt[:, t * 128 : (t + 1) * 128],
                        x_sb[:, i5 * 4 + t, kc * 128 : (kc + 1) * 128],
                        identf[:],
                    )
                precopy(xT[:, kc, i5 * 512 : (i5 + 1) * 512], pt[:])

        # ---- projections ----
        def proj_qk(m, i5):
            """m = 0,1: q head pairs (0,1),(2,3); m = 2,3: k head pairs."""
            pp = ps512.tile([128, 512], F32, tag="ps512", name=f"pp{m}_{i5}")
            for kc in range(4):
                nc.tensor.matmul(
                    pp[:],
                    wT[:, kc, m * 128 : (m + 1) * 128],
                    xT[:, kc, i5 * 512 : (i5 + 1) * 512],
                    start=(kc == 0),
                    stop=(kc == 3),
                )
            hp = m % 2
            if m < 2:
                nc.vector.tensor_copy(
                    qcT[:, hp, 1 + i5 * 512 : 1 + (i5 + 1) * 512], pp[:]
                )
                if i5 == 0:
                    nc.vector.tensor_copy(qcT[:, hp, 0:1], pp[:, 0:1])
            else:
                nc.vector.tensor_copy(kT[:, hp, i5 * 512 : (i5 + 1) * 512], pp[:])

        def proj_v(jc):
            pv = ps512.tile([128, 512], F32, tag="ps512", name=f"pv{jc}")
            for kc in range(4):
                nc.tensor.matmul(
                    pv[:, :RV],
                    xT[:, kc, jc * 128 : (jc + 1) * 128],
                    wT[:, kc, 2 * RV : 3 * RV],
                    start=(kc == 0),
                    stop=(kc == 3),
                )
            precopy(
                v1[:, jc, :].rearrange("p (h e) -> p h e", e=65)[:, :, 0:64],
                pv[:, :RV].rearrange("p (h e) -> p h e", e=64),
            )

        # i5-major: transpose x slice, then hp0 q/k proj and v proj for it,
        # so the first exp only waits on the i5=0 slice (~10us head)
        for i5 in range(NI5):
            transpose_x(i5)
            proj_qk(0, i5)  # q of hp 0
            proj_qk(2, i5)  # k of hp 0
            for jc in range(i5 * 4, i5 * 4 + 4):
                proj_v(jc)

        # ---- attention: flat pipeline over 8 (head-pair, i5) blocks ----
        # scores run two j-chunks ahead of attn@v; the next block's first two
        # score tiles are emitted before the current block's tail/epilogue so
        # the ScalarE exp stream never waits on PE program order.
        blocks = [(0, i5) for i5 in range(NI5)] + [(1, i5) for i5 in range(NI5)]
        n_blk = len(blocks)
        ot_tiles = {}
        ets = {}

        def alloc_ot(bi):
            ot_tiles[bi] = [
                ps_ot.tile([65, 512], F32, tag="ot", name=f"ot{bi}_{h}")
                for h in range(2)
            ]

        def scores(bi, jc):
            hp, i5 = blocks[bi]
            st = ps_st.tile([128, 1024], F32, tag="st", name=f"st{bi}_{jc}")
            nc.tensor.matmul(
                st[:, 0:512],
                kT[0:64, hp, jc * 128 : (jc + 1) * 128],
                qcT[0:64, hp, i5 * 512 : (i5 + 1) * 512],
                start=True,
                stop=True,
                tile_position=(0, 0),
            )
            nc.tensor.matmul(
                st[:, 512:1024],
                kT[64:128, hp, jc * 128 : (jc + 1) * 128],
                qcT[64:128, hp, i5 * 512 : (i5 + 1) * 512],
                start=True,
                stop=True,
                tile_position=(64, 0),
            )
            et = e_pool.tile([128, 1024], BF16, tag="et", name=f"et{bi}_{jc}")
            nc.scalar.activation(et[:], st[:], EXP, scale=SCALE)
            ets[(bi, jc)] = et

        def attnv(bi, jc):
            hp, i5 = blocks[bi]
            for h in range(2):
                ha = hp * 2 + h
                nc.tensor.matmul(
                    ot_tiles[bi][h][:],
                    v1[:, jc, ha * 65 : (ha + 1) * 65],
                    ets[(bi, jc)][:, h * 512 : (h + 1) * 512],
                    start=(jc == 0),
                    stop=(jc == NI - 1),
                )
            del ets[(bi, jc)]

        def epilogue(bi):
            hp, i5 = blocks[bi]
            for h in range(2):
                ha = hp * 2 + h
                ots = ot_pool.tile(
                    [65, 512], F32, tag="ots", name=f"ots{bi}_{h}"
                )
                nc.vector.tensor_copy(ots[:], ot_tiles[bi][h][:])
                for t in range(4):
                    ic = i5 * 4 + t
                    tr = ps512.tile(
                        [128, 512], F32, tag="ps512", name=f"tr{bi}_{h}_{t}"
                    )
                    nc.tensor.transpose(
                        tr[:, :65],
                        ots[:, t * 128 : (t + 1) * 128],
                        identf[0:65, 0:65],
                    )
                    rs = rs_pool.tile([128, 1], F32, tag="rs", name=f"rs{bi}_{h}_{t}")
                    nc.vector.reciprocal(rs[:], tr[:, 64:65])
                    nc.vector.tensor_scalar_mul(
                        out_sb[:, ic, ha * 64 : (ha + 1) * 64],
                        tr[:, 0:64],
                        rs[:],
                    )

        alloc_ot(0)
        scores(0, 0)
        scores(0, 1)
        for bi in range(n_blk):
            hp, i5 = blocks[bi]
            if hp == 0:
                # head pair 1's projection rides in PE slack under the
                # ACT-bound exp stream (emitted before any hp1 reads)
                proj_qk(1, i5)
                proj_qk(3, i5)
            for jc in range(2, NI):
                scores(bi, jc)
                attnv(bi, jc - 2)
            if bi + 1 < n_blk:
                alloc_ot(bi + 1)
                scores(bi + 1, 0)
            attnv(bi, NI - 2)
            if bi + 1 < n_blk:
                scores(bi + 1, 1)
            attnv(bi, NI - 1)
            epilogue(bi)
            if hp == 1:
                for t in range(4):
                    ic = i5 * 4 + t
                    nc.sync.dma_start(
                        o_d[ic * 128 : (ic + 1) * 128, :], out_sb[:, ic, :]
                    )

    return nc


def make_nc(n_tok: int = N) -> bass.Bass:
    nc = bacc.Bacc("TRN2", target_bir_lowering=False, debug=False)
    build_kernel(nc, n_tok=n_tok)
    nc.compile()
    return nc


def shard_inputs(x: np.ndarray, W_qkv: np.ndarray) -> list[dict]:
    """Core c = (b, g): b = c // 2, g = c % 2 (heads 4g..4g+3)."""
    in_maps = []
    for c in range(NCORES):
        b, g = divmod(c, 2)
        r0 = g * RV
        w_shard = np.concatenate(
            [
                W_qkv[r0 : r0 + RV],
                W_qkv[512 + r0 : 512 + r0 + RV],
                W_qkv[1024 + r0 : 1024 + r0 + RV],
            ],
            axis=0,
        )
        in_maps.append(
            {
                "x": np.ascontiguousarray(x[b], dtype=np.float32),
                "w": np.ascontiguousarray(w_shard, dtype=np.float32),
            }
        )
    return in_maps


def gather_outputs(results: list[dict]) -> np.ndarray:
    out = np.empty((B, N, H * DH), dtype=np.float32)
    for c in range(NCORES):
        b, g = divmod(c, 2)
        out[b, :, g * RV : (g + 1) * RV] = results[c]["o"]
    return out


_CACHED_NC = None


def kernel(x: np.ndarray, W_qkv: np.ndarray) -> np.ndarray:
    global _CACHED_NC
    from concourse.bass_utils import run_bass_kernel_spmd

    if _CACHED_NC is None:
        _CACHED_NC = make_nc()
    in_maps = shard_inputs(np.asarray(x), np.asarray(W_qkv))
    res = run_bass_kernel_spmd(_CACHED_NC, in_maps, core_ids=list(range(NCORES)))
    return gather_outputs(res.results)


if __name__ == "__main__":
    rng = np.random.default_rng(0)
    x = rng.standard_normal((B, N, D), dtype=np.float32)
    w = (rng.standard_normal((3 * H * DH, D), dtype=np.float32) * 0.02).astype(
        np.float32
    )
    out = kernel(x, w)
    print(out.shape, out.dtype)
